# revision 55
# baseline (speedup 1.0000x reference)
"""Trainium2 Bass kernel for single-head causal attention
(B=4, T=4096, E=1024, DH=128, fp32), sharded over 8 NeuronCores.

Sharding: 8 cores = 4 batches x 2 query-parity shards. Each core receives
the FULL pre-transposed bf16 X^T for its batch (columns ordered per window
as [4 own-parity token tiles | 4 partner-parity tiles]) and computes all
K^T/V projections locally — no collectives (the CC engine has ~50us of
fixed init latency plus ~11us semaphore-propagation on each end, which
dominated every exchange-based schedule). Queries are projected only for
the core's own-parity tiles.

Attention per 512-query group g runs over the 8(g+1) key tiles of windows
0..g, ordered [w0, w_g(diagonal), w1..w_{g-1}] so the first batch is
full-width/unmasked and the diagonal sits mid-group where its mask latency
hides. Diagonal tiles are column-narrowed to their live region (dead
columns are parity-independent); only the boundary 128x128 block is
masked (triangle for own-parity keys, 0/1-by-parity for partner keys —
both read from the masks input, which keeps the program core-uniform).

exp is batched 2 key tiles per ACT op (amortizes its ~352-cycle fixed
overhead). The softmax denominator comes from a DVE bf16 accumulation of
pt tiles plus one PE matmul per group (+2 direct matmuls for the final
batch, so the den matmul group never stalls the in-order PE on the DVE
tail); the den matmuls and epilogue are deferred into the next group's
first batch. All matmul operands are bf16 (fp32 PSUM accumulation).
"""

import numpy as np
import ml_dtypes

import concourse.bass as bass  # noqa: F401
import concourse.mybir as mybir
import concourse.tile as tile
from concourse import bacc
from concourse.bass_utils import run_bass_kernel_spmd

P = 128
B, T, E, DH = 4, 4096, 1024, 128
ECH = E // P            # 8 e-chunks
NW = T // (8 * P)       # 4 windows of 8 key tiles
NG = NW                 # 4 attention groups of 512 queries per core
NKT = T // P            # 32 key tiles
QG = 4 * P              # 512 queries per group
WT = 8 * P              # 1024 tokens per window
NCORES = 8
SCALE = 1.0 / np.sqrt(DH)

f32 = mybir.dt.float32
bf16 = mybir.dt.bfloat16
BF = ml_dtypes.bfloat16


def build_nc():
    nc = bacc.Bacc("TRN2", target_bir_lowering=False, debug=False,
                   num_devices=NCORES)
    xt_d = nc.dram_tensor("xt", [E, T], bf16, kind="ExternalInput").ap()
    wq = nc.dram_tensor("wq", [P, ECH * DH], bf16, kind="ExternalInput").ap()
    wk = nc.dram_tensor("wk", [P, ECH * DH], bf16, kind="ExternalInput").ap()
    wv = nc.dram_tensor("wv", [P, ECH * DH], bf16, kind="ExternalInput").ap()
    masks = nc.dram_tensor("masks", [2, P, P], bf16,
                           kind="ExternalInput").ap()
    cb = nc.dram_tensor("cb", [P, P + 1], bf16, kind="ExternalInput").ap()
    onef = nc.dram_tensor("onef", [1, 1], f32, kind="ExternalInput").ap()
    out = nc.dram_tensor("out", [T // 2, DH], f32, kind="ExternalOutput").ap()

    with tile.TileContext(nc) as tc:
        _emit(nc, tc, xt_d, wq, wk, wv, masks, cb, onef, out)
    nc.compile()
    return nc


def _emit(nc, tc, xt_d, wq, wk, wv, masks, cb, onef, out):
    import contextlib
    ctx = contextlib.ExitStack()
    with ctx:
        const = ctx.enter_context(tc.tile_pool(name="const", bufs=1))
        xt_pool = ctx.enter_context(tc.tile_pool(name="xt", bufs=2))
        kv_pool = ctx.enter_context(tc.tile_pool(name="kv", bufs=1))
        vtt_pool = ctx.enter_context(tc.tile_pool(name="vtt", bufs=2))
        pt_pool = ctx.enter_context(tc.tile_pool(name="pt", bufs=4))
        ps_pool = ctx.enter_context(tc.tile_pool(name="ps", bufs=2))
        osb_pool = ctx.enter_context(tc.tile_pool(name="osb", bufs=2))
        sm_pool = ctx.enter_context(tc.tile_pool(name="sm", bufs=6))
        st_psum = ctx.enter_context(
            tc.tile_pool(name="stp", bufs=2, space="PSUM"))
        scr_psum = ctx.enter_context(
            tc.tile_pool(name="scrp", bufs=2, space="PSUM"))
        avt_psum = ctx.enter_context(
            tc.tile_pool(name="avtp", bufs=1, space="PSUM"))
        den_psum = ctx.enter_context(
            tc.tile_pool(name="denp", bufs=1, space="PSUM"))

        # ---- small constants first, then first x^T window ----
        cb_sb = const.tile([P, P + 1], bf16)
        nc.sync.dma_start(cb_sb[:], cb[:])
        masks_sb = const.tile([P, 2, P], bf16)
        nc.sync.dma_start(masks_sb[:], masks.rearrange("j p c -> p j c"))
        identb = cb_sb[:, :P]
        ones_b = cb_sb[:, P:P + 1]
        one_f = const.tile([1, 1], f32)
        nc.sync.dma_start(one_f[:], onef[:])

        # interleave wk chunks with the xt0 chunks so the first
        # K-projection matmul can start as soon as chunk 0 lands
        xt0 = xt_pool.tile([P, ECH, WT], bf16, name="xt")
        w_sb = {}
        for name in ("wk", "wv", "wq"):
            w_sb[name] = const.tile([P, ECH * DH], bf16, name=f"{name}_sb")
        for ec in range(ECH):
            nc.sync.dma_start(w_sb["wk"][:, ec * DH:(ec + 1) * DH],
                              wk[:, ec * DH:(ec + 1) * DH])
            nc.sync.dma_start(xt0[:, ec, 0:QG],
                              xt_d[ec * P:(ec + 1) * P, 0:QG])
        nc.sync.dma_start(w_sb["wv"][:], wv[:])
        nc.sync.dma_start(w_sb["wq"][:], wq[:])
        for ec in range(ECH):
            nc.sync.dma_start(xt0[:, ec, QG:WT],
                              xt_d[ec * P:(ec + 1) * P, QG:WT])

        # pre-warm the ACT exp table set during the initial DMA wait
        act_w = sm_pool.tile([1, 1], f32, tag="aw", bufs=1, name="act_w")
        nc.scalar.activation(act_w[:], one_f[:],
                             mybir.ActivationFunctionType.Exp)

        # PE warmup during initial DMA wait
        warm = avt_psum.tile([P, QG], f32, tag="avt", name="warm")
        for _ in range(36):
            nc.tensor.matmul(warm[:, :P], identb[:], identb[:],
                             start=True, stop=True)

        kt_sb = kv_pool.tile([P, NKT * P], bf16)
        v_sb = kv_pool.tile([P, NKT * P], bf16)
        qt_sb = kv_pool.tile([P, NG * QG], bf16)

        # ---- projection work, queued as small PE "sections" that the
        # attention groups pump between windows (the attention pipeline is
        # ACT-paced: exp takes ~1.1us per 2-tile batch vs ~0.86us of PE
        # work, so the PE has slack to absorb the projections) ----
        from collections import deque
        sections = deque()
        prep_left = {}
        xts = {}

        def _proj_section(w, h, wname, dst_cb):
            """Two 4-matmul accumulation sections (+ evac) for one
            projection over half-window h. Safe to split because nothing
            else allocates from the scr ring between consecutive pumped
            sections (the epilogue uses the den pool for its PSUM tiles)."""
            st = {}

            def first():
                xh = xts[w][:, :, h * QG:(h + 1) * QG]
                pp = scr_psum.tile([P, QG], f32, tag="scr", name="pp")
                st["pp"] = pp
                for ec in range(4):
                    nc.tensor.matmul(
                        pp[:], w_sb[wname][:, ec * DH:(ec + 1) * DH],
                        xh[:, ec, :], start=(ec == 0), stop=False)

            def second():
                xh = xts[w][:, :, h * QG:(h + 1) * QG]
                pp = st["pp"]
                for ec in range(4, ECH):
                    nc.tensor.matmul(
                        pp[:], w_sb[wname][:, ec * DH:(ec + 1) * DH],
                        xh[:, ec, :], start=False, stop=(ec == ECH - 1))
                dst_cb(pp)

            return [first, second]

        def queue_prep(w):
            if w == 0:
                xt = xt0
            else:
                xt = xt_pool.tile([P, ECH, WT], bf16, name="xt")
                for ec in range(ECH):
                    nc.sync.dma_start(
                        xt[:, ec, :],
                        xt_d[ec * P:(ec + 1) * P, WT * w:WT * (w + 1)])
            xts[w] = xt

            secs = []
            for h in range(2):
                hs = slice(WT * w + h * QG, WT * w + (h + 1) * QG)

                def k_evac(pp, hs=hs):
                    nc.vector.tensor_copy(kt_sb[:, hs], pp[:])

                secs += _proj_section(w, h, "wk", k_evac)

                vtt_box = {}

                def v_evac(pp, vtt_box=vtt_box):
                    vtt = vtt_pool.tile([P, QG], bf16, name="vtt")
                    nc.vector.tensor_copy(vtt[:], pp[:])
                    vtt_box["t"] = vtt

                secs += _proj_section(w, h, "wv", v_evac)

                if h == 0:
                    def q_evac(pp, w=w):
                        nc.vector.tensor_copy(qt_sb[:, QG * w:QG * (w + 1)],
                                              pp[:])

                    secs += _proj_section(w, h, "wq", q_evac)

                def transp(vtt_box=vtt_box, hs=hs):
                    vtt = vtt_box["t"]
                    vnp = den_psum.tile([P, QG], bf16, tag="natp", bufs=1,
                                        name="vnp")
                    for kb in range(4):
                        nc.tensor.transpose(
                            vnp[:, kb * P:(kb + 1) * P],
                            vtt[:, kb * P:(kb + 1) * P],
                            identb[:])
                    nc.vector.tensor_copy(v_sb[:, hs], vnp[:])

                secs.append(transp)
            for s in secs:
                sections.append((w, s))
            prep_left[w] = len(secs)

        def pump_one():
            if sections:
                w, s = sections.popleft()
                s()
                prep_left[w] -= 1

        def pump_until(w):
            while sections and prep_left.get(w, 0) > 0:
                pump_one()

        def attn_group(g, after_first_batch=None):
            """One 512-query group over the 8(g+1) key tiles of windows
            0..g. Returns (avt, den, ptsum-ish, finish) with finish()
            emitting the den matmuls + epilogue; the caller flushes it
            during the next group."""
            n = 8 * (g + 1)
            qt_g = qt_sb[:, QG * g:QG * (g + 1)]
            avt = avt_psum.tile([P, QG], f32, tag="avt", name="avt")
            ptsum = ps_pool.tile([P, QG], bf16, name="ptsum")
            windows = [0] if g == 0 else [0, g] + list(range(1, g))
            tiles = [(w, j) for w in windows for j in range(8)]
            pump_until(g)
            for b in range(n // 2):
                w0, j0 = tiles[2 * b]
                if b > 0 and b % 3 == 0:
                    # absorb one queued projection section into the
                    # ACT-paced pipeline's PE slack (~250ns/batch deficit)
                    pump_one()
                diag = (w0 == g)
                c_lo = (j0 % 4) * P if diag else 0
                st2 = st_psum.tile([P, 2, QG], f32, name="st2")
                for i in range(2):
                    kc = (8 * w0 + j0 + i) * P
                    # write from c_lo (not the tile's own live start) so
                    # the batched exp below never reads unwritten PSUM
                    nc.tensor.matmul(st2[:, i, c_lo:], kt_sb[:, kc:kc + P],
                                     qt_g[:, c_lo:], start=True, stop=True)
                pt2 = pt_pool.tile([P, 2, QG], bf16, name="pt2")
                nc.scalar.activation(pt2[:, :, c_lo:], st2[:, :, c_lo:],
                                     mybir.ActivationFunctionType.Exp,
                                     scale=SCALE)
                if b == 0 and after_first_batch is not None:
                    # previous group's epilogue lands here: its avt-bank
                    # evac runs on the DVE while this batch's exp is still
                    # in flight, so the first avt matmul below never stalls
                    after_first_batch()
                if diag:
                    for i in range(2):
                        j = j0 + i
                        bs = slice((j % 4) * P, (j % 4 + 1) * P)
                        nc.vector.tensor_mul(pt2[:, i, bs], pt2[:, i, bs],
                                             masks_sb[:, j // 4, :])
                for i in range(2):
                    idx = 2 * b + i
                    c0 = ((j0 + i) % 4) * P if diag else 0
                    kc = (8 * w0 + j0 + i) * P
                    nc.tensor.matmul(avt[:, c0:], v_sb[:, kc:kc + P],
                                     pt2[:, i, c0:],
                                     start=(idx == 0), stop=(idx == n - 1))
                # DVE-accumulate pt for the softmax denominator (the den
                # matmuls are deferred into the next group, so this chain's
                # tail is never on the PE critical path)
                for i in range(2):
                    c0 = ((j0 + i) % 4) * P if diag else 0
                    if 2 * b + i == 0:
                        nc.vector.tensor_copy(ptsum[:], pt2[:, 0, :])
                    else:
                        nc.vector.tensor_add(ptsum[:, c0:], ptsum[:, c0:],
                                             pt2[:, i, c0:])

            def finish():
                # epilogue first: the avt evac frees the avt bank before
                # the next group's first avt matmul needs it
                avt_sb = osb_pool.tile([P, QG], bf16, tag="asb",
                                       name="avt_sb")
                nc.vector.tensor_copy(avt_sb[:], avt[:])
                # den, directly transposed: den^T[q] = sum_tok ptsum[tok,q]
                # via 4 matmuls with the ptsum subtile as the stationary
                # operand and the ones-column as the (N=1) moving operand
                dent = den_psum.tile([P, 4], f32, tag="natp", bufs=1,
                                     name="dent")
                for a in range(4):
                    nc.tensor.matmul(dent[:, a:a + 1],
                                     ptsum[:, a * P:(a + 1) * P],
                                     ones_b[:], start=True, stop=True)
                recip = sm_pool.tile([P, 4], f32, name="recip")
                nc.vector.reciprocal(recip[:], dent[:])

                natp = den_psum.tile([P, QG], bf16, tag="natp", bufs=1,
                                     name="natp")
                for a in range(4):
                    nc.tensor.transpose(
                        natp[:, a * P:(a + 1) * P],
                        avt_sb[:, a * P:(a + 1) * P],
                        identb[:])
                out_sb = osb_pool.tile([P, QG], f32, tag="osb",
                                       name="out_sb")
                for a in range(4):
                    # alternate DVE/ACT so the four scales run pairwise
                    # in parallel instead of serially on one engine
                    if a % 2 == 0:
                        nc.vector.tensor_scalar_mul(
                            out_sb[:, a * DH:(a + 1) * DH],
                            natp[:, a * DH:(a + 1) * DH],
                            recip[:, a:a + 1])
                    else:
                        nc.scalar.mul(out_sb[:, a * DH:(a + 1) * DH],
                                      natp[:, a * DH:(a + 1) * DH],
                                      recip[:, a:a + 1])
                out_r = out[QG * g:QG * (g + 1), :].rearrange(
                    "(a q) d -> q a d", a=4)
                osr = out_sb[:].rearrange("q (a d) -> q a d", a=4)
                for a in range(4):
                    # per-subtile DMA so the store overlaps the muls above
                    nc.sync.dma_start(out_r[:, a:a + 1, :],
                                      osr[:, a:a + 1, :])

            return finish

        pending = [None]

        def run_pending():
            if pending[0] is not None:
                pending[0]()
                pending[0] = None

        for w in range(NW):
            queue_prep(w)
        pending[0] = attn_group(0)
        pending[0] = attn_group(1, run_pending)
        pending[0] = attn_group(2, run_pending)
        pending[0] = attn_group(3, run_pending)
        run_pending()


# ---------------- host side ----------------

def _tile_order(p):
    """Per-window token-tile order: 4 own-parity tiles then 4 partner."""
    return np.array([8 * w + q + 2 * a
                     for w in range(NW)
                     for q in (p, 1 - p)
                     for a in range(4)])


def _masks(p):
    """Boundary-block masks for the diagonal window: [0] = causal triangle
    (own-parity key tile s vs query subtile a==s), [1] = partner-parity
    boundary (key tile true offset 1-p+2s vs query p+2s: all-live iff
    p==1)."""
    m = np.zeros((2, P, P), np.float32)
    kl = np.arange(P)[:, None]
    ql = np.arange(P)[None, :]
    m[0][kl <= ql] = 1.0
    if p == 1:
        m[1] = 1.0
    return m


_NC_CACHE = []


def _get_nc():
    if not _NC_CACHE:
        _NC_CACHE.append(build_nc())
    return _NC_CACHE[0]


def _run(norm_inputs, Wq, Wk, Wv, **spmd_kwargs):
    nc = _get_nc()
    xf = np.asarray(norm_inputs, np.float32)
    wqb = np.ascontiguousarray(
        np.asarray(Wq, np.float32).reshape(ECH, P, DH).transpose(1, 0, 2)
    ).astype(BF).reshape(P, ECH * DH)
    wkb = np.ascontiguousarray(
        np.asarray(Wk, np.float32).reshape(ECH, P, DH).transpose(1, 0, 2)
    ).astype(BF).reshape(P, ECH * DH)
    wvb = np.ascontiguousarray(
        np.asarray(Wv, np.float32).reshape(ECH, P, DH).transpose(1, 0, 2)
    ).astype(BF).reshape(P, ECH * DH)
    cbh = np.zeros((P, P + 1), np.float32)
    cbh[:, :P] = np.eye(P)
    cbh[:, P] = 1.0
    cbh = cbh.astype(BF)
    onefh = np.ones((1, 1), np.float32)
    in_maps = []
    for c in range(NCORES):
        b, p = c // 2, c % 2
        xp = xf[b].reshape(NKT, P, E)[_tile_order(p)].reshape(T, E)
        xtp = np.ascontiguousarray(xp.T).astype(BF)
        in_maps.append({
            "xt": xtp, "wq": wqb, "wk": wkb, "wv": wvb,
            "masks": _masks(p).astype(BF),
            "cb": cbh, "onef": onefh,
        })
    res = run_bass_kernel_spmd(nc, in_maps, core_ids=list(range(NCORES)),
                               **spmd_kwargs)
    outf = np.empty((B, T, DH), np.float32)
    for c in range(NCORES):
        b, p = c // 2, c % 2
        oc = res.results[c]["out"].reshape(NG, 4, P, DH)
        full = outf[b].reshape(NKT, P, DH)
        for i in range(NG):
            for a in range(4):
                full[8 * i + p + 2 * a] = oc[i, a]
    return outf, res


def kernel(norm_inputs, Wq, Wk, Wv):
    outf, _ = _run(norm_inputs, Wq, Wk, Wv)
    return outf


# revision 57
# speedup vs baseline: 1.0126x; 1.0126x over previous
"""Trainium2 Bass kernel for single-head causal attention
(B=4, T=4096, E=1024, DH=128, fp32), sharded over 8 NeuronCores.

Sharding: 8 cores = 4 batches x 2 query-parity shards. Each core receives
the FULL pre-transposed bf16 X^T for its batch (columns ordered per window
as [4 own-parity token tiles | 4 partner-parity tiles]) and computes all
K^T/V projections locally — no collectives (the CC engine has ~50us of
fixed init latency plus ~11us semaphore-propagation on each end, which
dominated every exchange-based schedule). Queries are projected only for
the core's own-parity tiles.

Attention per 512-query group g runs over the 8(g+1) key tiles of windows
0..g, ordered [w0, w_g(diagonal), w1..w_{g-1}] so the first batch is
full-width/unmasked and the diagonal sits mid-group where its mask latency
hides. Diagonal tiles are column-narrowed to their live region (dead
columns are parity-independent); only the boundary 128x128 block is
masked (triangle for own-parity keys, 0/1-by-parity for partner keys —
both read from the masks input, which keeps the program core-uniform).

exp is batched 2 key tiles per ACT op (amortizes its ~352-cycle fixed
overhead). The softmax denominator comes from a DVE bf16 accumulation of
pt tiles plus one PE matmul per group (+2 direct matmuls for the final
batch, so the den matmul group never stalls the in-order PE on the DVE
tail); the den matmuls and epilogue are deferred into the next group's
first batch. All matmul operands are bf16 (fp32 PSUM accumulation).
"""

import numpy as np
import ml_dtypes

import concourse.bass as bass  # noqa: F401
import concourse.mybir as mybir
import concourse.tile as tile
from concourse import bacc
from concourse.bass_utils import run_bass_kernel_spmd

P = 128
B, T, E, DH = 4, 4096, 1024, 128
ECH = E // P            # 8 e-chunks
NW = T // (8 * P)       # 4 windows of 8 key tiles
NG = NW                 # 4 attention groups of 512 queries per core
NKT = T // P            # 32 key tiles
QG = 4 * P              # 512 queries per group
WT = 8 * P              # 1024 tokens per window
NCORES = 8
SCALE = 1.0 / np.sqrt(DH)

f32 = mybir.dt.float32
bf16 = mybir.dt.bfloat16
BF = ml_dtypes.bfloat16


def build_nc():
    nc = bacc.Bacc("TRN2", target_bir_lowering=False, debug=False,
                   num_devices=NCORES)
    xt_d = nc.dram_tensor("xt", [E, T], bf16, kind="ExternalInput").ap()
    wq = nc.dram_tensor("wq", [P, ECH * DH], bf16, kind="ExternalInput").ap()
    wk = nc.dram_tensor("wk", [P, ECH * DH], bf16, kind="ExternalInput").ap()
    wv = nc.dram_tensor("wv", [P, ECH * DH], bf16, kind="ExternalInput").ap()
    masks = nc.dram_tensor("masks", [2, P, P], bf16,
                           kind="ExternalInput").ap()
    cb = nc.dram_tensor("cb", [P, P + 1], bf16, kind="ExternalInput").ap()
    onef = nc.dram_tensor("onef", [1, 1], f32, kind="ExternalInput").ap()
    out = nc.dram_tensor("out", [T // 2, DH], f32, kind="ExternalOutput").ap()

    with tile.TileContext(nc) as tc:
        _emit(nc, tc, xt_d, wq, wk, wv, masks, cb, onef, out)
    nc.compile()
    return nc


def _emit(nc, tc, xt_d, wq, wk, wv, masks, cb, onef, out):
    import contextlib
    ctx = contextlib.ExitStack()
    with ctx:
        const = ctx.enter_context(tc.tile_pool(name="const", bufs=1))
        xt_pool = ctx.enter_context(tc.tile_pool(name="xt", bufs=2))
        kv_pool = ctx.enter_context(tc.tile_pool(name="kv", bufs=1))
        vtt_pool = ctx.enter_context(tc.tile_pool(name="vtt", bufs=2))
        pt_pool = ctx.enter_context(tc.tile_pool(name="pt", bufs=4))
        ps_pool = ctx.enter_context(tc.tile_pool(name="ps", bufs=2))
        osb_pool = ctx.enter_context(tc.tile_pool(name="osb", bufs=2))
        sm_pool = ctx.enter_context(tc.tile_pool(name="sm", bufs=6))
        st_psum = ctx.enter_context(
            tc.tile_pool(name="stp", bufs=2, space="PSUM"))
        scr_psum = ctx.enter_context(
            tc.tile_pool(name="scrp", bufs=2, space="PSUM"))
        avt_psum = ctx.enter_context(
            tc.tile_pool(name="avtp", bufs=1, space="PSUM"))
        den_psum = ctx.enter_context(
            tc.tile_pool(name="denp", bufs=1, space="PSUM"))

        # ---- small constants first, then first x^T window ----
        cb_sb = const.tile([P, P + 1], bf16)
        nc.sync.dma_start(cb_sb[:], cb[:])
        masks_sb = const.tile([P, 2, P], bf16)
        nc.sync.dma_start(masks_sb[:], masks.rearrange("j p c -> p j c"))
        identb = cb_sb[:, :P]
        ones_b = cb_sb[:, P:P + 1]
        one_f = const.tile([1, 1], f32)
        nc.sync.dma_start(one_f[:], onef[:])

        # interleave wk chunks with the xt0 chunks so the first
        # K-projection matmul can start as soon as chunk 0 lands
        xt0 = xt_pool.tile([P, ECH, WT], bf16, name="xt")
        w_sb = {}
        for name in ("wk", "wv", "wq"):
            w_sb[name] = const.tile([P, ECH * DH], bf16, name=f"{name}_sb")
        for ec in range(ECH):
            nc.sync.dma_start(w_sb["wk"][:, ec * DH:(ec + 1) * DH],
                              wk[:, ec * DH:(ec + 1) * DH])
            nc.sync.dma_start(xt0[:, ec, 0:QG],
                              xt_d[ec * P:(ec + 1) * P, 0:QG])
        nc.sync.dma_start(w_sb["wv"][:], wv[:])
        nc.sync.dma_start(w_sb["wq"][:], wq[:])
        for ec in range(ECH):
            nc.sync.dma_start(xt0[:, ec, QG:WT],
                              xt_d[ec * P:(ec + 1) * P, QG:WT])

        # pre-warm the ACT exp table set during the initial DMA wait
        act_w = sm_pool.tile([1, 1], f32, tag="aw", bufs=1, name="act_w")
        nc.scalar.activation(act_w[:], one_f[:],
                             mybir.ActivationFunctionType.Exp)

        # PE warmup during initial DMA wait
        warm = avt_psum.tile([P, QG], f32, tag="avt", name="warm")
        for _ in range(20):
            nc.tensor.matmul(warm[:, :P], identb[:], identb[:],
                             start=True, stop=True)

        kt_sb = kv_pool.tile([P, NKT * P], bf16)
        v_sb = kv_pool.tile([P, NKT * P], bf16)
        qt_sb = kv_pool.tile([P, NG * QG], bf16)

        # ---- projection work, queued as small PE "sections" that the
        # attention groups pump between windows (the attention pipeline is
        # ACT-paced: exp takes ~1.1us per 2-tile batch vs ~0.86us of PE
        # work, so the PE has slack to absorb the projections) ----
        from collections import deque
        sections = deque()
        prep_left = {}
        xts = {}

        def _proj_section(w, h, wname, dst_cb):
            """Two 4-matmul accumulation sections (+ evac) for one
            projection over half-window h. Safe to split because nothing
            else allocates from the scr ring between consecutive pumped
            sections (the epilogue uses the den pool for its PSUM tiles)."""
            st = {}

            def first():
                xh = xts[w][:, :, h * QG:(h + 1) * QG]
                pp = scr_psum.tile([P, QG], f32, tag="scr", name="pp")
                st["pp"] = pp
                for ec in range(4):
                    nc.tensor.matmul(
                        pp[:], w_sb[wname][:, ec * DH:(ec + 1) * DH],
                        xh[:, ec, :], start=(ec == 0), stop=False)

            def second():
                xh = xts[w][:, :, h * QG:(h + 1) * QG]
                pp = st["pp"]
                for ec in range(4, ECH):
                    nc.tensor.matmul(
                        pp[:], w_sb[wname][:, ec * DH:(ec + 1) * DH],
                        xh[:, ec, :], start=False, stop=(ec == ECH - 1))
                dst_cb(pp)

            return [first, second]

        def queue_prep(w):
            if w == 0:
                xt = xt0
            else:
                xt = xt_pool.tile([P, ECH, WT], bf16, name="xt")
                for ec in range(ECH):
                    nc.sync.dma_start(
                        xt[:, ec, :],
                        xt_d[ec * P:(ec + 1) * P, WT * w:WT * (w + 1)])
            xts[w] = xt

            secs = []
            for h in range(2):
                hs = slice(WT * w + h * QG, WT * w + (h + 1) * QG)

                def k_evac(pp, hs=hs):
                    nc.vector.tensor_copy(kt_sb[:, hs], pp[:])

                secs += _proj_section(w, h, "wk", k_evac)

                vtt_box = {}

                def v_evac(pp, vtt_box=vtt_box):
                    vtt = vtt_pool.tile([P, QG], bf16, name="vtt")
                    nc.vector.tensor_copy(vtt[:], pp[:])
                    vtt_box["t"] = vtt

                secs += _proj_section(w, h, "wv", v_evac)

                if h == 0:
                    def q_evac(pp, w=w):
                        nc.vector.tensor_copy(qt_sb[:, QG * w:QG * (w + 1)],
                                              pp[:])

                    secs += _proj_section(w, h, "wq", q_evac)

                def transp(vtt_box=vtt_box, hs=hs):
                    vtt = vtt_box["t"]
                    vnp = den_psum.tile([P, QG], bf16, tag="natp", bufs=1,
                                        name="vnp")
                    for kb in range(4):
                        nc.tensor.transpose(
                            vnp[:, kb * P:(kb + 1) * P],
                            vtt[:, kb * P:(kb + 1) * P],
                            identb[:])
                    nc.vector.tensor_copy(v_sb[:, hs], vnp[:])

                secs.append(transp)
            for s in secs:
                sections.append((w, s))
            prep_left[w] = len(secs)

        def pump_one():
            if sections:
                w, s = sections.popleft()
                s()
                prep_left[w] -= 1

        def pump_until(w):
            while sections and prep_left.get(w, 0) > 0:
                pump_one()

        def attn_group(g, after_first_batch=None):
            """One 512-query group over the 8(g+1) key tiles of windows
            0..g. Returns (avt, den, ptsum-ish, finish) with finish()
            emitting the den matmuls + epilogue; the caller flushes it
            during the next group."""
            n = 8 * (g + 1)
            qt_g = qt_sb[:, QG * g:QG * (g + 1)]
            avt = avt_psum.tile([P, QG], f32, tag="avt", name="avt")
            ptsum = ps_pool.tile([P, QG], bf16, name="ptsum")
            windows = [0] if g == 0 else [0, g] + list(range(1, g))
            tiles = [(w, j) for w in windows for j in range(8)]
            pump_until(g)
            for b in range(n // 2):
                w0, j0 = tiles[2 * b]
                if b > 0 and b % 3 == 0:
                    # absorb one queued projection section into the
                    # ACT-paced pipeline's PE slack (~250ns/batch deficit)
                    pump_one()
                diag = (w0 == g)
                c_lo = (j0 % 4) * P if diag else 0
                st2 = st_psum.tile([P, 2, QG], f32, name="st2")
                for i in range(2):
                    kc = (8 * w0 + j0 + i) * P
                    # write from c_lo (not the tile's own live start) so
                    # the batched exp below never reads unwritten PSUM
                    nc.tensor.matmul(st2[:, i, c_lo:], kt_sb[:, kc:kc + P],
                                     qt_g[:, c_lo:], start=True, stop=True)
                pt2 = pt_pool.tile([P, 2, QG], bf16, name="pt2")
                nc.scalar.activation(pt2[:, :, c_lo:], st2[:, :, c_lo:],
                                     mybir.ActivationFunctionType.Exp,
                                     scale=SCALE)
                if b == 0 and after_first_batch is not None:
                    # previous group's epilogue lands here: its avt-bank
                    # evac runs on the DVE while this batch's exp is still
                    # in flight, so the first avt matmul below never stalls
                    after_first_batch()
                if diag:
                    for i in range(2):
                        j = j0 + i
                        bs = slice((j % 4) * P, (j % 4 + 1) * P)
                        nc.vector.tensor_mul(pt2[:, i, bs], pt2[:, i, bs],
                                             masks_sb[:, j // 4, :])
                for i in range(2):
                    idx = 2 * b + i
                    c0 = ((j0 + i) % 4) * P if diag else 0
                    kc = (8 * w0 + j0 + i) * P
                    nc.tensor.matmul(avt[:, c0:], v_sb[:, kc:kc + P],
                                     pt2[:, i, c0:],
                                     start=(idx == 0), stop=(idx == n - 1))
                # DVE-accumulate pt for the softmax denominator (the den
                # matmuls are deferred into the next group, so this chain's
                # tail is never on the PE critical path)
                for i in range(2):
                    c0 = ((j0 + i) % 4) * P if diag else 0
                    if 2 * b + i == 0:
                        nc.vector.tensor_copy(ptsum[:], pt2[:, 0, :])
                    else:
                        nc.vector.tensor_add(ptsum[:, c0:], ptsum[:, c0:],
                                             pt2[:, i, c0:])

            def finish():
                # epilogue first: the avt evac frees the avt bank before
                # the next group's first avt matmul needs it
                avt_sb = osb_pool.tile([P, QG], bf16, tag="asb",
                                       name="avt_sb")
                nc.vector.tensor_copy(avt_sb[:], avt[:])
                # den, directly transposed: den^T[q] = sum_tok ptsum[tok,q]
                # via 4 matmuls with the ptsum subtile as the stationary
                # operand and the ones-column as the (N=1) moving operand
                dent = den_psum.tile([P, 4], f32, tag="natp", bufs=1,
                                     name="dent")
                for a in range(4):
                    nc.tensor.matmul(dent[:, a:a + 1],
                                     ptsum[:, a * P:(a + 1) * P],
                                     ones_b[:], start=True, stop=True)
                recip = sm_pool.tile([P, 4], f32, name="recip")
                nc.vector.reciprocal(recip[:], dent[:])

                natp = den_psum.tile([P, QG], bf16, tag="natp", bufs=1,
                                     name="natp")
                for a in range(4):
                    nc.tensor.transpose(
                        natp[:, a * P:(a + 1) * P],
                        avt_sb[:, a * P:(a + 1) * P],
                        identb[:])
                out_sb = osb_pool.tile([P, QG], f32, tag="osb",
                                       name="out_sb")
                for a in range(4):
                    nc.vector.tensor_scalar_mul(
                        out_sb[:, a * DH:(a + 1) * DH],
                        natp[:, a * DH:(a + 1) * DH],
                        recip[:, a:a + 1])
                out_r = out[QG * g:QG * (g + 1), :].rearrange(
                    "(a q) d -> q a d", a=4)
                osr = out_sb[:].rearrange("q (a d) -> q a d", a=4)
                for a in range(4):
                    # per-subtile DMA so the store overlaps the muls above
                    nc.sync.dma_start(out_r[:, a:a + 1, :],
                                      osr[:, a:a + 1, :])

            return finish

        pending = [None]

        def run_pending():
            if pending[0] is not None:
                pending[0]()
                pending[0] = None

        for w in range(NW):
            queue_prep(w)
        pending[0] = attn_group(0)
        pending[0] = attn_group(1, run_pending)
        pending[0] = attn_group(2, run_pending)
        pending[0] = attn_group(3, run_pending)
        run_pending()


# ---------------- host side ----------------

def _tile_order(p):
    """Per-window token-tile order: 4 own-parity tiles then 4 partner."""
    return np.array([8 * w + q + 2 * a
                     for w in range(NW)
                     for q in (p, 1 - p)
                     for a in range(4)])


def _masks(p):
    """Boundary-block masks for the diagonal window: [0] = causal triangle
    (own-parity key tile s vs query subtile a==s), [1] = partner-parity
    boundary (key tile true offset 1-p+2s vs query p+2s: all-live iff
    p==1)."""
    m = np.zeros((2, P, P), np.float32)
    kl = np.arange(P)[:, None]
    ql = np.arange(P)[None, :]
    m[0][kl <= ql] = 1.0
    if p == 1:
        m[1] = 1.0
    return m


_NC_CACHE = []


def _get_nc():
    if not _NC_CACHE:
        _NC_CACHE.append(build_nc())
    return _NC_CACHE[0]


def _run(norm_inputs, Wq, Wk, Wv, **spmd_kwargs):
    nc = _get_nc()
    xf = np.asarray(norm_inputs, np.float32)
    wqb = np.ascontiguousarray(
        np.asarray(Wq, np.float32).reshape(ECH, P, DH).transpose(1, 0, 2)
    ).astype(BF).reshape(P, ECH * DH)
    wkb = np.ascontiguousarray(
        np.asarray(Wk, np.float32).reshape(ECH, P, DH).transpose(1, 0, 2)
    ).astype(BF).reshape(P, ECH * DH)
    wvb = np.ascontiguousarray(
        np.asarray(Wv, np.float32).reshape(ECH, P, DH).transpose(1, 0, 2)
    ).astype(BF).reshape(P, ECH * DH)
    cbh = np.zeros((P, P + 1), np.float32)
    cbh[:, :P] = np.eye(P)
    cbh[:, P] = 1.0
    cbh = cbh.astype(BF)
    onefh = np.ones((1, 1), np.float32)
    in_maps = []
    for c in range(NCORES):
        b, p = c // 2, c % 2
        xp = xf[b].reshape(NKT, P, E)[_tile_order(p)].reshape(T, E)
        xtp = np.ascontiguousarray(xp.T).astype(BF)
        in_maps.append({
            "xt": xtp, "wq": wqb, "wk": wkb, "wv": wvb,
            "masks": _masks(p).astype(BF),
            "cb": cbh, "onef": onefh,
        })
    res = run_bass_kernel_spmd(nc, in_maps, core_ids=list(range(NCORES)),
                               **spmd_kwargs)
    outf = np.empty((B, T, DH), np.float32)
    for c in range(NCORES):
        b, p = c // 2, c % 2
        oc = res.results[c]["out"].reshape(NG, 4, P, DH)
        full = outf[b].reshape(NKT, P, DH)
        for i in range(NG):
            for a in range(4):
                full[8 * i + p + 2 * a] = oc[i, a]
    return outf, res


def kernel(norm_inputs, Wq, Wk, Wv):
    outf, _ = _run(norm_inputs, Wq, Wk, Wv)
    return outf
